# revision 3
# baseline (speedup 1.0000x reference)
"""Trainium2 Bass kernel for the controlled-U (CU) gate application.

Math: the reference builds U = P0 (x) I (x) ... + P1 (x) Mexp (x) I ...
with dim=2, wires=12, index=(0,1), control_state=(1,). This factors as

    U = diag(I_2048, Mexp (x) I_1024)        (4096 x 4096)

so U @ x is:
    out[0:2048]     = x[0:2048]                        (identity)
    out[2048:3072]  = c00 * x[2048:3072] + c01 * x[3072:4096]
    out[3072:4096]  = c10 * x[2048:3072] + c11 * x[3072:4096]

with [[c00, c01], [c10, c11]] = Mexp = expm(M - M^H), a 2x2 unitary
computed exactly on host (eigendecomposition of the 2x2 Hermitian
generator).

The identity block is pure data movement with zero arithmetic, so it is
handled in the host-side gather: the top 2048 output rows are assembled
directly from the input array while interleaving to complex64 (the host
touches every element there anyway).  The device computes only the
non-trivial part -- the 2x2 complex mix over the bottom 2048 rows --
sharded row-wise across the 8 cores (128 pair-rows each).

Device-side formulation: each core packs its slice as four 128-partition
tiles where partitions carry 32-row groups of (x1_re, x1_im, x2_re,
x2_im) for paired rows (x1 from the c00/c01 block, x2 from the c10/c11
block).  The whole complex 2x2 mix is then ONE bf16 matmul per
512-column PSUM bank with a single constant stationary

    V = kron(Q, I_32),   Q[a, b] = coefficient of input group a
                                   in output group b

(out = V^T @ x contracts the partition dim, mixing re/im and the two
row blocks in one pass).  8 matmuls + 1 stationary load total; PSUM
banks are evacuated to bf16 SBUF split between the ACT and DVE engines,
and the 4 tile stores alternate the two HWDGE rings so loads and stores
overlap.  Per-core HBM traffic is ~2.06 MiB (vs 4.2 MiB when the
identity block rode the device), and the bf16 pipeline measures
~4e-3 rel err against the 2e-2 gate.
"""

import ml_dtypes
import numpy as np

import concourse.bacc as bacc
import concourse.mybir as mybir
from concourse.tile import TileContext
from concourse.bass_utils import run_bass_kernel_spmd

# Problem geometry (hardcoded per the task contract).
D = 4096           # state dimension 2**12
B = 1024           # batch
NCORES = 8
P = 128            # SBUF partitions
PROWS = D // 4 // NCORES   # 128 bottom pair rows per core
NT = 4             # tiles per core: 4 x [128, 1024]
CH = 512           # PSUM bank = 512 f32 columns
G = 32             # rows per partition group (4 groups of 32 = 128)
F32 = mybir.dt.float32
BF16 = mybir.dt.bfloat16
NPBF = ml_dtypes.bfloat16

N_WARM = 8         # dummy matmuls to unthrottle the PE clock gate


def _build_nc() -> bacc.Bacc:
    """Build the per-core Bass/Tile program (identical on all 8 cores)."""
    nc = bacc.Bacc("TRN2", enable_partition_id=False)

    v_in = nc.dram_tensor("V", [P, P], BF16, kind="ExternalInput")
    x_in = nc.dram_tensor("X", [P, NT * B], BF16, kind="ExternalInput")
    y_out = nc.dram_tensor("Y", [P, NT * B], BF16, kind="ExternalOutput")

    with TileContext(nc) as tc:
        with (
            tc.tile_pool(name="const", bufs=1) as const_pool,
            tc.tile_pool(name="io", bufs=1) as io_pool,
            tc.tile_pool(name="scr", bufs=1) as scr_pool,
            tc.tile_pool(name="psum", bufs=1, space="PSUM") as psum_pool,
        ):
            v_sb = const_pool.tile([P, P], BF16, tag="v")
            x_sb = io_pool.tile([P, NT * B], BF16, tag="x")
            y_sb = io_pool.tile([P, NT * B], BF16, tag="y")

            # Loads, split across the two HWDGE rings.  DMA transfers
            # effectively serialize through the shared DMA-engine pool at
            # ~360 GB/s, so what matters is descriptor-ready order: ring
            # SP carries tiles 0/1, ring ACT carries V (tiny) then tiles
            # 2/3, which makes pool arrival order == consumption order
            # (V, X0, X1, X2, X3) while both rings' issue costs overlap.
            nc.sync.dma_start(x_sb[:, 0 * B : 1 * B], x_in[:, 0 * B : 1 * B])
            nc.scalar.dma_start(v_sb[:], v_in[:])
            nc.sync.dma_start(x_sb[:, 1 * B : 2 * B], x_in[:, 1 * B : 2 * B])
            nc.scalar.dma_start(x_sb[:, 2 * B : 3 * B], x_in[:, 2 * B : 3 * B])
            nc.scalar.dma_start(x_sb[:, 3 * B : 4 * B], x_in[:, 3 * B : 4 * B])

            # PE warmup: the HAM clock gate runs the PE at 1.2 GHz until
            # it has seen ~3.5 us of sustained activity; dummy matmuls on
            # a memset tile bridge the load-latency window (first tile is
            # consumable ~3.5 us in) so payload matmuls run at 2.4 GHz.
            # They write a payload PSUM bank -- harmless, the payload
            # matmul resets it with start=True.
            dummy = scr_pool.tile([P, CH], BF16, tag="dummy")
            nc.gpsimd.memset(dummy[:], 0.0)
            warm_ps = psum_pool.tile([P, CH], F32, tag="ps0", name="ps0w")
            for _ in range(N_WARM):
                nc.tensor.matmul(warm_ps[:], dummy[:, 0:P], dummy[:],
                                 start=True, stop=True)

            # Payload: per tile, one matmul per 512-col half (its own
            # PSUM bank, single start/stop); ACT evacuates even banks,
            # DVE odd banks (bf16 cast).  Stores go PER BANK, right after
            # that bank's evac, alternating rings: the ~1.9 us HWDGE
            # issue latency of store k then overlaps bank k+1's matmul/
            # evac instead of stacking after the whole tile.
            for t in range(NT):
                for h in range(2):
                    k = 2 * t + h
                    cs = slice(t * B + h * CH, t * B + (h + 1) * CH)
                    pt = psum_pool.tile([P, CH], F32, tag=f"ps{k}",
                                        name=f"ps{k}")
                    nc.tensor.matmul(pt[:], v_sb[:], x_sb[:, cs],
                                     start=True, stop=True)
                    if h == 0:
                        nc.scalar.copy(y_sb[:, cs], pt[:])
                        nc.sync.dma_start(y_out[:, cs], y_sb[:, cs])
                    else:
                        nc.vector.tensor_copy(y_sb[:, cs], pt[:])
                        nc.scalar.dma_start(y_out[:, cs], y_sb[:, cs])

    nc.finalize()
    return nc


_NC_CACHE = None


def _get_nc() -> bacc.Bacc:
    global _NC_CACHE
    if _NC_CACHE is None:
        _NC_CACHE = _build_nc()
    return _NC_CACHE


def _coef_values(M_re: np.ndarray, M_im: np.ndarray):
    """Host-side 2x2 expm of the anti-Hermitian generator -> V stationary.

    Returns (V, None): V is the [128, 128] bf16 kron(Q, I_32) stationary
    (second slot kept for interface compat with older harnesses).
    """
    M = M_re.astype(np.float64) + 1j * M_im.astype(np.float64)
    A = M - M.conj().T          # anti-Hermitian
    H = -1j * A                 # Hermitian
    w, Vec = np.linalg.eigh(H)
    Mexp = Vec @ np.diag(np.exp(1j * w)) @ Vec.conj().T   # expm(A), exact
    cr, ci = Mexp.real, Mexp.imag
    # out = V^T @ x with partition groups (x1re, x1im, x2re, x2im) and
    # output groups (o1re, o1im, o2re, o2im): V[p, i] = Q[p//32, i//32].
    Q = np.array([
        [cr[0, 0],  ci[0, 0],  cr[1, 0],  ci[1, 0]],
        [-ci[0, 0], cr[0, 0], -ci[1, 0],  cr[1, 0]],
        [cr[0, 1],  ci[0, 1],  cr[1, 1],  ci[1, 1]],
        [-ci[0, 1], cr[0, 1], -ci[1, 1],  cr[1, 1]],
    ], dtype=np.float32)
    V = np.kron(Q, np.eye(G, dtype=np.float32)).astype(NPBF)
    return V, None


def _in_map(x_re, x_im, V, cvec, d: int) -> dict:
    """Per-core input dict: pack the core's 128 pair-rows as 4 tiles of
    [128, 1024] with partition groups (x1re, x1im, x2re, x2im) x 32."""
    b1 = D // 2 + d * PROWS
    b2 = 3 * D // 4 + d * PROWS

    def grp(a, b0):
        return np.asarray(a[b0 : b0 + PROWS], dtype=NPBF).reshape(NT, G, B)

    # [NT, 4*G, B] -> [4*G, NT, B] -> [128, 4096]
    X = np.concatenate(
        [grp(x_re, b1), grp(x_im, b1), grp(x_re, b2), grp(x_im, b2)],
        axis=1,
    ).transpose(1, 0, 2).reshape(P, NT * B)
    return {"V": V, "X": np.ascontiguousarray(X)}


def kernel(M_re, M_im, x_re, x_im) -> np.ndarray:
    M_re = np.asarray(M_re, dtype=np.float32)
    M_im = np.asarray(M_im, dtype=np.float32)
    x_re = np.ascontiguousarray(x_re, dtype=np.float32)
    x_im = np.ascontiguousarray(x_im, dtype=np.float32)

    V, _ = _coef_values(M_re, M_im)
    in_maps = [_in_map(x_re, x_im, V, None, d) for d in range(NCORES)]

    nc = _get_nc()
    res = run_bass_kernel_spmd(nc, in_maps, core_ids=list(range(NCORES)))

    full = np.empty((D, B), dtype=np.complex64)
    # Identity block: assembled straight from the input during the gather.
    full.real[: D // 2] = x_re[: D // 2]
    full.imag[: D // 2] = x_im[: D // 2]
    for d, r in enumerate(res.results):
        b1 = D // 2 + d * PROWS
        b2 = 3 * D // 4 + d * PROWS
        Y = np.asarray(r["Y"]).reshape(P, NT, B).transpose(1, 0, 2)
        full.real[b1 : b1 + PROWS] = Y[:, 0 * G : 1 * G].reshape(PROWS, B)
        full.imag[b1 : b1 + PROWS] = Y[:, 1 * G : 2 * G].reshape(PROWS, B)
        full.real[b2 : b2 + PROWS] = Y[:, 2 * G : 3 * G].reshape(PROWS, B)
        full.imag[b2 : b2 + PROWS] = Y[:, 3 * G : 4 * G].reshape(PROWS, B)
    return full
